# revision 41
# baseline (speedup 1.0000x reference)
"""Neural ODE (RK4, 2-layer MLP dynamics) Trainium2 Bass kernel.

Strategy: data-parallel over 8 NeuronCores (batch 4096 -> 512/core).
On-chip layout is transposed: hT = [H=256, B=512], column block k in {0,1}
= H-rows [128k, 128k+128). The per-core batch is split into 2 halves of
256 columns (b=0,1) that pipeline through the engines with a half-stage
skew.

Matmul operands are float32r (relaxed-precision fp32, same bytes): the PE
streams f32r at 1 cycle/row vs 4 for strict fp32. The integration state h
is strict fp32, updated only by vector adds; matmuls read it via bitcast.

Fused critical path: the next stage's input is produced in ONE DVE op
straight from PSUM, tmp_j = c_j*(W2@z_j) + h (scalar_tensor_tensor).
The b2 bias of the inner layer is folded into the NEXT L1's bias by
linearity (z = relu(W1@tmp + b1 + c_j*(W1@b2))), so no u-eviction or
h-add sits between L2 and the next stage's L1 matmuls.

The RK4 combine uses linearity instead of per-stage accumulation:
k1+2k2+2k3+k4 = W2@(z1+2z2+2z3) + W2@z4. zsum123 = (z1+z2) + (2z3+z2)
is built incrementally off the critical path (acc1 = z1+z2 on GPSIMD at
stage 1, acc2 = 2z3+z2 one DVE stt at stage 2, final add on GPSIMD), and
stage 3 runs two 4-matmul W2 groups into one PSUM bank per half.
s = (dtm/6)*pb4 + dt*b2 evicts with fused scale+bias; h' = h + s at full
fp32 (hn), plus a parallel f32r-rounded copy hr = h + s that feeds the
next step's matmuls (the BIR verifier requires matmul f32r inputs to be
produced rounded). Per-step output projection W_out @ hr -> [64, B] is
DMA'd out (pso eviction deferred to stage 1 to keep the ACT/DVE queues
clear); the host transposes back and adds b_out.

Scheduling: outproj matmuls fill the stage-0 PE slots; emission order
keeps each engine queue in data-arrival order; z evictions all on ACT so
the DVE queue stays dedicated to the latency-critical tmp stts.

PSUM note: matmul start=True clears the has_written bits of the ENTIRE
bank; start=False matmuls overwrite fresh regions and accumulate written
ones. Banks: pA m0/m1 (2) + pB m0/m1 (2+2) + shared pso/pb4 pool (2) = 8.
"""

import numpy as np

HIDDEN = 256
OUT = 64
BATCH = 4096
TSTEPS = 100
NCORES = 8
BC = BATCH // NCORES  # 512 batch per core
HB = BC // 2  # 256, half-batch (free dim of most ops)
P = 128

_cache = {}


ENG = {  # engine assignment knobs (sim-tuned)
    "z_b0m0": "act", "z_b0m1": "act", "z_b1m0": "act", "z_b1m1": "act",
    "s_b0": "dve", "s_b1": "act",
    "hn_k0": "gps", "hn_k1": "dve",
    "hr_k0": "dve", "hr_k1": "dve",
    "acc1": "gps", "zsum": "dve",
    "osb_b0": "act", "osb_b1": "act",
    "opb1_pos": "mid",
}


def _build(dts, dtm, eng=None, repeat=1):
    """Build the Bass kernel. dts: 99 python-float step sizes, dtm: mean dt
    (used for the identity-injection matrices and host-folded biases)."""
    import concourse.bass as bass
    import concourse.mybir as mybir
    from contextlib import ExitStack
    from concourse.bacc import Bacc
    from concourse.tile import TileContext

    f32 = mybir.dt.float32
    f32r = mybir.dt.float32r
    AF = mybir.ActivationFunctionType
    ALU = mybir.AluOpType

    E = dict(ENG)
    if eng:
        E.update(eng)

    nc = Bacc("TRN2", target_bir_lowering=False, debug=False)

    xT = nc.dram_tensor("xT", [OUT, BC], f32r, kind="ExternalInput")
    winT_d = nc.dram_tensor("winT", [OUT, HIDDEN], f32r, kind="ExternalInput")
    w1T_d = nc.dram_tensor("w1T", [P, 512], f32r, kind="ExternalInput")
    w2T_d = nc.dram_tensor("w2T", [P, 512], f32r, kind="ExternalInput")
    woutT_d = nc.dram_tensor("woutT", [P, 128], f32r, kind="ExternalInput")
    ident_d = nc.dram_tensor("ident", [P, 384], f32r, kind="ExternalInput")
    bias_d = nc.dram_tensor("biases", [P, 10], f32, kind="ExternalInput")
    b2r_d = nc.dram_tensor("b2row", [1, 256], f32r, kind="ExternalInput")
    one_d = nc.dram_tensor("onerow", [1, 256], f32r, kind="ExternalInput")
    out_d = nc.dram_tensor("out", [TSTEPS, OUT, BC], f32, kind="ExternalOutput")

    nsteps = len(dts)  # 99

    with TileContext(nc) as tc, ExitStack() as ctx:
        const = ctx.enter_context(tc.tile_pool(name="const", bufs=1))
        hpool = ctx.enter_context(tc.tile_pool(name="hpool", bufs=2))
        hrpool = ctx.enter_context(tc.tile_pool(name="hrpool", bufs=2))
        zpool = ctx.enter_context(tc.tile_pool(name="zpool", bufs=4))
        zspool = ctx.enter_context(tc.tile_pool(name="zspool", bufs=2))
        tpool = ctx.enter_context(tc.tile_pool(name="tpool", bufs=4))
        spool = ctx.enter_context(tc.tile_pool(name="spool", bufs=2))
        opool = ctx.enter_context(tc.tile_pool(name="opool", bufs=4))
        # PSUM: pA0/pA1 + pB0/pB1 + pso/pb4 shared
        pa = ctx.enter_context(
            tc.tile_pool(name="pa", bufs=int(E.get("pa_bufs", 1)), space="PSUM")
        )
        pbp = ctx.enter_context(
            tc.tile_pool(name="pbp", bufs=int(E.get("pb_bufs", 2)), space="PSUM")
        )
        p4p = ctx.enter_context(tc.tile_pool(name="p4p", bufs=2, space="PSUM"))

        # ---- load constants into SBUF
        x_sb = const.tile([OUT, BC], f32r, name="x_sb")
        win = const.tile([OUT, HIDDEN], f32r, name="win")
        w1 = const.tile([P, 512], f32r, name="w1")
        w2 = const.tile([P, 512], f32r, name="w2")
        wout = const.tile([P, 128], f32r, name="wout")
        ident = const.tile([P, 384], f32r, name="ident")
        bia = const.tile([P, 10], f32, name="bia")
        b2row = const.tile([1, 256], f32r, name="b2row")
        onerow = const.tile([1, 256], f32r, name="onerow")
        nc.sync.dma_start(x_sb[:], xT[:, :])
        nc.sync.dma_start(win[:], winT_d[:, :])
        nc.sync.dma_start(w1[:], w1T_d[:, :])
        nc.sync.dma_start(w2[:], w2T_d[:, :])
        nc.sync.dma_start(wout[:], woutT_d[:, :])
        nc.sync.dma_start(ident[:], ident_d[:, :])
        nc.sync.dma_start(bia[:], bias_d[:, :])
        nc.sync.dma_start(b2row[:], b2r_d[:, :])
        nc.sync.dma_start(onerow[:], one_d[:, :])

        # PE matmuls may carry at most ONE sync wait; absorb every const-DMA
        # queue tick into the PE vector clock up front via dummy 1x1 matmuls.
        dmy = pa.tile([1, 1], f32, tag="pA0", name="dmy")
        for cst in (x_sb, win, w1, w2, wout, ident, bia, b2row, onerow):
            c1 = cst[:, 0:1].bitcast(f32)  # f32r 1x1 matmul is invalid ISA
            nc.tensor.matmul(
                dmy[:], c1, c1, start=True, stop=True, skip_group_check=True
            )

        I2 = ident[:, 0:128]  # (2/dtm) I
        I4 = ident[:, 128:256]  # (4/dtm) I

        def bcol(j):  # [128,1] bias column
            return bia[:, j : j + 1]

        # bias cols (q = W1@b2): 0,1 b_in; 2,3 b1 (z@j0);
        # 4,5 b1+(dtm/2)q (z@j1,j2); 6,7 dtm*b2 (s evict); 8,9 b1+dtm*q (z@j3)
        ZB = (2, 4, 4, 8)  # z bias col base per stage

        def wblk(w, k, m):  # W1T/W2T block (k, m)
            j = (k * 2 + m) * 128
            return w[:, j : j + 128]

        def new_h(b):
            return hpool.tile([P, 2 * HB], f32, tag=f"hb{b}", name="h")

        def new_hr(b):
            return hrpool.tile([P, 2 * HB], f32r, tag=f"hrb{b}", name="hr")

        def kv(hh_b, k):  # k-chunk view of a per-half tile
            return hh_b[:, k * HB : (k + 1) * HB]

        tte = {"dve": nc.vector, "gps": nc.gpsimd}

        # ---- h0 = W_in @ xT + b_in   (full batch, N=512)
        h = [new_h(0), new_h(1)]
        hr = [new_hr(0), new_hr(1)]
        for m in range(2):
            ps = pa.tile([P, BC], f32, tag=f"pA{m}", name="ps_init")
            nc.tensor.matmul(
                ps[:], win[:, m * 128 : (m + 1) * 128], x_sb[:], start=True, stop=True
            )
            for b in range(2):
                src = ps[:, b * HB : (b + 1) * HB]
                if b == 0:
                    nc.scalar.activation(
                        kv(h[b], m), src, AF.Identity, bias=bcol(m), scale=1.0
                    )
                else:
                    nc.vector.tensor_scalar(
                        kv(h[b], m), src, bcol(m), None, op0=ALU.add
                    )
        for b in range(2):
            nc.vector.tensor_copy(hr[b][:], h[b][:])

        def emit_op_mm(h_b, b):
            pso = p4p.tile([OUT, HB], f32, tag="p4", name="pso")
            for k in range(2):
                nc.tensor.matmul(
                    pso[:], wout[:, k * 64 : (k + 1) * 64], kv(h_b, k),
                    start=(k == 0), stop=(k == 1),
                )
            return pso

        def emit_osb(t, pso, b):
            # pso eviction + DMA, deferred so it never sits ahead of
            # latency-critical z/tmp ops in the ACT/DVE queues
            osb = opool.tile([OUT, HB], f32, tag=f"osb{b}", name="osb")
            if E[f"osb_b{b}"] == "act":
                nc.scalar.copy(osb[:], pso[:])
            else:
                nc.vector.tensor_copy(osb[:], pso[:])
            nc.sync.dma_start(out_d[t, :, b * HB : (b + 1) * HB], osb[:])

        def emit_outproj(t, h_b, b):
            emit_osb(t, emit_op_mm(h_b, b), b)

        def emit_L1(b, iv):
            pA = [
                pa.tile([P, HB], f32, tag="pA0", name="pAt"),
                pa.tile([P, HB], f32, tag="pA1", name="pAt"),
            ]
            for k, m in ((0, 0), (0, 1), (1, 0), (1, 1)):
                nc.tensor.matmul(
                    pA[m][:], wblk(w1, k, m), iv[k],
                    start=(k == 0), stop=(k == 1),
                    skip_group_check=True,
                )
            return pA

        def emit_z(b, j, pA):
            z = [
                zpool.tile([P, HB], f32r, tag=f"z{j}0", name="z0"),
                zpool.tile([P, HB], f32r, tag=f"z{j}1", name="z1"),
            ]
            for m in range(2):
                engn = E.get(f"z_j{j}_b{b}m{m}", E[f"z_b{b}m{m}"])
                bc = bcol(ZB[j] + m)
                if engn == "act":
                    nc.scalar.activation(
                        z[m][:], pA[m][:], AF.Relu, bias=bc, scale=1.0
                    )
                else:
                    nc.vector.tensor_scalar(
                        z[m][:], pA[m][:], bc, 0.0, op0=ALU.add, op1=ALU.max
                    )
            return z

        def emit_L2(b, z):
            pB = [
                pbp.tile([P, HB], f32, tag="pB0", name="pBt"),
                pbp.tile([P, HB], f32, tag="pB1", name="pBt"),
            ]
            for k, m in ((0, 0), (0, 1), (1, 0), (1, 1)):
                nc.tensor.matmul(
                    pB[m][:], wblk(w2, k, m), z[k][:],
                    start=(k == 0), stop=(k == 1),
                    skip_group_check=True,
                )
            return pB

        def emit_tmp(b, j, pB, dt):
            # tmp = c_j*(W2@z_j) + h in ONE DVE op from PSUM (critical path)
            c = (dt / 2.0, dt / 2.0, dt)[j]
            tmp = [
                tpool.tile([P, HB], f32r, tag="tmp0", name="t0"),
                tpool.tile([P, HB], f32r, tag="tmp1", name="t1"),
            ]
            for m in range(2):
                nc.vector.scalar_tensor_tensor(
                    tmp[m][:], pB[m][:], c, kv(h[b], m).bitcast(f32r),
                    op0=ALU.mult, op1=ALU.add,
                )
            return tmp

        def emit_acc1(b, zs):
            # zsum123 = z1+2z2+2z3 = (z1+z2) + (2z3+z2), spread over stages:
            # acc1 = z1+z2 at j1 (plain add, idle GPSIMD), acc2 = 2z3+z2 at
            # j2 (one DVE stt), final add on GPSIMD. Exact RK4 weights.
            acc = []
            for m in range(2):
                a = zspool.tile([P, HB], f32r, tag=f"a1{m}", name="a1")
                tte[E["acc1"]].tensor_tensor(
                    a[:], zs[0][b][m][:], zs[1][b][m][:], op=ALU.add
                )
                acc.append(a)
            return acc

        def emit_acc2(b, zs):
            acc = []
            for m in range(2):
                a = zspool.tile([P, HB], f32r, tag=f"a2{m}", name="a2")
                nc.vector.scalar_tensor_tensor(
                    a[:], zs[2][b][m][:], 2.0, zs[1][b][m][:],
                    op0=ALU.mult, op1=ALU.add,
                )
                acc.append(a)
            return acc

        def emit_zsum(b, acc1, acc2):
            zsum = []
            for m in range(2):
                zsm = zspool.tile([P, HB], f32r, tag=f"zs{m}", name="zsm")
                tte[E["zsum"]].tensor_tensor(
                    zsm[:], acc1[b][m][:], acc2[b][m][:], op=ALU.add
                )
                zsum.append(zsm)
            return zsum

        def emit_w2grp(b, zin, pb4, first):
            # 4-matmul W2 group accumulating into pb4[b]
            for k, m in ((0, 0), (0, 1), (1, 0), (1, 1)):
                nc.tensor.matmul(
                    pb4[b][m][:], wblk(w2, k, m), zin[k][:],
                    start=(first and k == 0 and m == 0),
                    stop=(not first and k == 1 and m == 1),
                    skip_group_check=True,
                )

        def emit_s_hn(b, hn, hrn, h):
            sc = dtm / 6.0
            sl = [
                spool.tile([P, HB], f32, tag="s0", name="s0"),
                spool.tile([P, HB], f32, tag="s1", name="s1"),
            ]

            def s_op(m):
                if E.get(f"s_b{b}m{m}", E[f"s_b{b}"]) == "act":
                    nc.scalar.activation(
                        sl[m][:], pb4_cur[b][m][:], AF.Identity,
                        bias=bcol(6 + m), scale=sc,
                    )
                else:
                    nc.vector.tensor_scalar(
                        sl[m][:], pb4_cur[b][m][:], sc, bcol(6 + m),
                        op0=ALU.mult, op1=ALU.add,
                    )

            def hr_op(kq):
                tte[E[f"hr_k{kq}"]].tensor_tensor(
                    kv(hrn[b], kq), kv(h[b], kq).bitcast(f32r),
                    sl[kq][:].bitcast(f32r), op=ALU.add,
                )

            if E.get("bnd_order", "sshh") == "shsh":
                s_op(0); hr_op(0); s_op(1); hr_op(1)
            else:
                s_op(0); s_op(1); hr_op(0); hr_op(1)
            for kq in range(2):
                tte[E[f"hn_k{kq}"]].tensor_tensor(
                    kv(hn[b], kq), kv(h[b], kq), sl[kq][:], op=ALU.add
                )

        # ---- time stepping (repeat>1 is a timing-only mode)
        for t in range(nsteps * repeat):
            dt = dts[t % nsteps]
            hn = [new_h(0), new_h(1)]
            hrn = [new_hr(0), new_hr(1)]
            inp = [
                [kv(hr[0], 0), kv(hr[0], 1)],
                [kv(hr[1], 0), kv(hr[1], 1)],
            ]
            pb4 = [None, None]
            pb4_cur = pb4

            pso = [None, None]
            zs = [[None, None] for _ in range(4)]  # zs[j][b]
            acc1 = [None, None]
            acc2 = [None, None]
            zsum = [None, None]
            for j in range(3):  # RK4 stages 0-2
                # alt_halves: swap which half leads on odd stages so the
                # trailing-half latency penalty alternates instead of
                # always hitting b1
                ba, bb = (1, 0) if (E.get("alt_halves", 0) and j % 2) else (0, 1)
                pAa = emit_L1(ba, inp[ba])
                zs[j][ba] = emit_z(ba, j, pAa)
                # -- slot after L1(ba)+z: guaranteed-ready PE work
                if j == 0 and not E.get("op_at_j1", 0):
                    pso[ba] = emit_op_mm(hr[ba], ba)
                    if E["opb1_pos"] == "early":
                        pso[bb] = emit_op_mm(hr[bb], bb)

                pAb = emit_L1(bb, inp[bb])
                zs[j][bb] = emit_z(bb, j, pAb)
                if j == 1 and E.get("op_at_j1", 0):
                    pso[0] = emit_op_mm(hr[0], 0)
                    pso[1] = emit_op_mm(hr[1], 1)

                pBa = emit_L2(ba, zs[j][ba])
                tmpa = emit_tmp(ba, j, pBa, dt)
                inp[ba] = [tmpa[0][:], tmpa[1][:]]
                if j == 2:
                    # acc2 behind tmp in the DVE queue: tmp is critical
                    acc2[ba] = emit_acc2(ba, zs)
                # -- slot after L2(ba)
                if j == 0 and not E.get("op_at_j1", 0) and E["opb1_pos"] == "mid":
                    pso[bb] = emit_op_mm(hr[bb], bb)
                if j == 2:
                    zsum[ba] = emit_zsum(ba, acc1, acc2)
                pBb = emit_L2(bb, zs[j][bb])
                tmpb = emit_tmp(bb, j, pBb, dt)
                inp[bb] = [tmpb[0][:], tmpb[1][:]]
                if j == 0 and not E.get("op_at_j1", 0) and E["opb1_pos"] == "end":
                    pso[bb] = emit_op_mm(hr[bb], bb)
                if j == 1:
                    # deferred pso evictions: quiet queue point
                    emit_osb(t % nsteps, pso[0], 0)
                    emit_osb(t % nsteps, pso[1], 1)
                    acc1[0] = emit_acc1(0, zs)
                    acc1[1] = emit_acc1(1, zs)
                if j == 2:
                    acc2[bb] = emit_acc2(bb, zs)
                    zsum[bb] = emit_zsum(bb, acc1, acc2)

            # ---- stage 3: W2 groups accumulate pb4, then s/hr/hn
            pA0 = emit_L1(0, inp[0])
            zs[3][0] = emit_z(0, 3, pA0)
            for b in range(2):
                pt = p4p.tile([P, 2 * HB], f32, tag="p4", name="pb4")
                pb4[b] = [pt[:, 0:HB], pt[:, HB : 2 * HB]]
            pb4_cur = pb4
            emit_w2grp(0, zsum[0], pb4, first=True)
            if E.get("j3_order", "abab") == "aabb":
                # b0's full boundary chain first: its s->hr starts earliest
                emit_w2grp(0, zs[3][0], pb4, first=False)
                emit_s_hn(0, hn, hrn, h)
                pA1 = emit_L1(1, inp[1])
                zs[3][1] = emit_z(1, 3, pA1)
                emit_w2grp(1, zsum[1], pb4, first=True)
                emit_w2grp(1, zs[3][1], pb4, first=False)
                emit_s_hn(1, hn, hrn, h)
            else:
                pA1 = emit_L1(1, inp[1])
                zs[3][1] = emit_z(1, 3, pA1)
                emit_w2grp(0, zs[3][0], pb4, first=False)
                emit_s_hn(0, hn, hrn, h)
                emit_w2grp(1, zsum[1], pb4, first=True)
                emit_w2grp(1, zs[3][1], pb4, first=False)
                emit_s_hn(1, hn, hrn, h)
            h = hn
            hr = hrn

        # final output projection (t = nsteps)
        for b in range(2):
            emit_outproj(nsteps, hr[b], b)

    nc.compile()  # bacc passes: event-sem legalization, reg alloc, DCE
    return nc


def _prep_shared(W_in, b_in, W1, b1, W2, b2, W_out, dtm):
    f = np.float32

    def pack_blocks(WT):  # [256,256] -> [128, 512] blocks (k*2+m)
        blks = [
            WT[k * 128 : (k + 1) * 128, m * 128 : (m + 1) * 128]
            for k in range(2)
            for m in range(2)
        ]
        return np.ascontiguousarray(np.concatenate(blks, axis=1), dtype=f)

    winT = np.ascontiguousarray(W_in.T, dtype=f)  # [64, 256]
    w1T = pack_blocks(W1.T.astype(f))
    w2T = pack_blocks(W2.T.astype(f))
    wt = W_out.T.astype(f)  # [256, 64]
    woutT = np.ascontiguousarray(
        np.concatenate([wt[0:128, :], wt[128:256, :]], axis=1), dtype=f
    )  # [128, 128]

    I = np.eye(128, dtype=f)
    ident = np.ascontiguousarray(
        np.concatenate(
            [(f(2.0) / dtm) * I, (f(4.0) / dtm) * I, (f(6.0) / dtm) * I], axis=1
        ),
        dtype=f,
    )

    def cols2(v):  # [256] -> two [128] cols
        return [v[0:128], v[128:256]]

    b1 = b1.astype(f)
    b2 = b2.astype(f)
    q = (W1.astype(f) @ b2).astype(f)  # W1 @ b2, [256]
    cols = (
        cols2(b_in.astype(f))
        + cols2(b1)                       # z bias @ j0
        + cols2(b1 + (dtm / f(2.0)) * q)  # z bias @ j1, j2
        + cols2(dtm * b2)                 # s-evict bias
        + cols2(b1 + dtm * q)             # z bias @ j3
    )
    biases = np.ascontiguousarray(np.stack(cols, axis=1), dtype=f)  # [128, 10]
    b2row = np.ascontiguousarray((f(6.0) * b2).reshape(1, 256), dtype=f)
    onerow = np.ones((1, 256), dtype=f)
    return dict(winT=winT, w1T=w1T, w2T=w2T, woutT=woutT, ident=ident,
                biases=biases, b2row=b2row, onerow=onerow)


_last_results = None


def kernel(x, t_span, W_in, b_in, W1, b1, W2, b2, W_out, b_out):
    global _last_results
    from concourse.bass_utils import run_bass_kernel_spmd

    f = np.float32
    x = np.asarray(x, f)
    t_span = np.asarray(t_span, f)
    dts = np.diff(t_span).astype(f)
    dtm = f(dts.mean())

    key = dts.tobytes()
    if key not in _cache:
        _cache[key] = _build([float(d) for d in dts], float(dtm))
    nc = _cache[key]

    shared = _prep_shared(
        np.asarray(W_in), np.asarray(b_in), np.asarray(W1), np.asarray(b1),
        np.asarray(W2), np.asarray(b2), np.asarray(W_out), dtm,
    )
    in_maps = []
    for c in range(NCORES):
        xc = np.ascontiguousarray(x[c * BC : (c + 1) * BC].T, dtype=f)  # [64, 512]
        m = dict(shared)
        m["xT"] = xc
        in_maps.append(m)

    res = run_bass_kernel_spmd(nc, in_maps, core_ids=list(range(NCORES)))
    _last_results = res
    outs = [np.asarray(r["out"]) for r in res.results]  # each [100, 64, 512]
    full = np.concatenate([o.transpose(0, 2, 1) for o in outs], axis=1)
    full = full + np.asarray(b_out, f)[None, None, :]
    return np.ascontiguousarray(full, dtype=f)


# revision 42
# speedup vs baseline: 1.0116x; 1.0116x over previous
"""Neural ODE (RK4, 2-layer MLP dynamics) Trainium2 Bass kernel.

Strategy: data-parallel over 8 NeuronCores (batch 4096 -> 512/core).
On-chip layout is transposed: hT = [H=256, B=512], column block k in {0,1}
= H-rows [128k, 128k+128). The per-core batch is split into 2 halves of
256 columns (b=0,1) that pipeline through the engines with a half-stage
skew.

Matmul operands are float32r (relaxed-precision fp32, same bytes): the PE
streams f32r at 1 cycle/row vs 4 for strict fp32. The integration state h
is strict fp32, updated only by vector adds; matmuls read it via bitcast.

Fused critical path: the next stage's input is produced in ONE DVE op
straight from PSUM, tmp_j = c_j*(W2@z_j) + h (scalar_tensor_tensor).
The b2 bias of the inner layer is folded into the NEXT L1's bias by
linearity (z = relu(W1@tmp + b1 + c_j*(W1@b2))), so no u-eviction or
h-add sits between L2 and the next stage's L1 matmuls.

The RK4 combine uses linearity instead of per-stage accumulation:
k1+2k2+2k3+k4 = W2@(z1+2z2+2z3) + W2@z4. zsum123 = (z1+z2) + (2z3+z2)
is built incrementally off the critical path (acc1 = z1+z2 on GPSIMD at
stage 1, acc2 = 2z3+z2 one DVE stt at stage 2, final add on GPSIMD), and
stage 3 runs two 4-matmul W2 groups into one PSUM bank per half.
s = (dtm/6)*pb4 + dt*b2 evicts with fused scale+bias; h' = h + s at full
fp32 (hn), plus a parallel f32r-rounded copy hr = h + s that feeds the
next step's matmuls (the BIR verifier requires matmul f32r inputs to be
produced rounded). Per-step output projection W_out @ hr -> [64, B] is
DMA'd out (pso eviction deferred to stage 1 to keep the ACT/DVE queues
clear); the host transposes back and adds b_out.

Scheduling: outproj matmuls fill the stage-0 PE slots; emission order
keeps each engine queue in data-arrival order; z evictions all on ACT so
the DVE queue stays dedicated to the latency-critical tmp stts.

PSUM note: matmul start=True clears the has_written bits of the ENTIRE
bank; start=False matmuls overwrite fresh regions and accumulate written
ones. Banks: pA m0/m1 (2) + pB m0/m1 (2+2) + shared pso/pb4 pool (2) = 8.
"""

import numpy as np

HIDDEN = 256
OUT = 64
BATCH = 4096
TSTEPS = 100
NCORES = 8
BC = BATCH // NCORES  # 512 batch per core
HB = BC // 2  # 256, half-batch (free dim of most ops)
P = 128

_cache = {}


ENG = {  # engine assignment knobs (sim-tuned)
    "z_b0m0": "act", "z_b0m1": "act", "z_b1m0": "act", "z_b1m1": "act",
    "s_b0": "dve", "s_b1": "act",
    "hn_k0": "gps", "hn_k1": "dve",
    "hr_k0": "dve", "hr_k1": "dve",
    "acc1": "gps", "zsum": "dve",
    "osb_b0": "act", "osb_b1": "act",
    "opb1_pos": "mid",
}


def _build(dts, dtm, eng=None, repeat=1):
    """Build the Bass kernel. dts: 99 python-float step sizes, dtm: mean dt
    (used for the identity-injection matrices and host-folded biases)."""
    import concourse.bass as bass
    import concourse.mybir as mybir
    from contextlib import ExitStack
    from concourse.bacc import Bacc
    from concourse.tile import TileContext

    f32 = mybir.dt.float32
    f32r = mybir.dt.float32r
    AF = mybir.ActivationFunctionType
    ALU = mybir.AluOpType

    E = dict(ENG)
    if eng:
        E.update(eng)

    nc = Bacc("TRN2", target_bir_lowering=False, debug=False)

    xT = nc.dram_tensor("xT", [OUT, BC], f32r, kind="ExternalInput")
    winT_d = nc.dram_tensor("winT", [OUT, HIDDEN], f32r, kind="ExternalInput")
    w1T_d = nc.dram_tensor("w1T", [P, 512], f32r, kind="ExternalInput")
    w2T_d = nc.dram_tensor("w2T", [P, 512], f32r, kind="ExternalInput")
    woutT_d = nc.dram_tensor("woutT", [P, 128], f32r, kind="ExternalInput")
    ident_d = nc.dram_tensor("ident", [P, 384], f32r, kind="ExternalInput")
    bias_d = nc.dram_tensor("biases", [P, 10], f32, kind="ExternalInput")
    b2r_d = nc.dram_tensor("b2row", [1, 256], f32r, kind="ExternalInput")
    one_d = nc.dram_tensor("onerow", [1, 256], f32r, kind="ExternalInput")
    out_d = nc.dram_tensor("out", [TSTEPS, OUT, BC], f32, kind="ExternalOutput")

    nsteps = len(dts)  # 99

    with TileContext(nc) as tc, ExitStack() as ctx:
        const = ctx.enter_context(tc.tile_pool(name="const", bufs=1))
        hpool = ctx.enter_context(tc.tile_pool(name="hpool", bufs=2))
        hrpool = ctx.enter_context(tc.tile_pool(name="hrpool", bufs=2))
        zpool = ctx.enter_context(tc.tile_pool(name="zpool", bufs=4))
        zspool = ctx.enter_context(tc.tile_pool(name="zspool", bufs=2))
        tpool = ctx.enter_context(tc.tile_pool(name="tpool", bufs=4))
        spool = ctx.enter_context(tc.tile_pool(name="spool", bufs=2))
        opool = ctx.enter_context(tc.tile_pool(name="opool", bufs=4))
        # PSUM: pA0/pA1 + pB0/pB1 + pso/pb4 shared
        pa = ctx.enter_context(
            tc.tile_pool(name="pa", bufs=int(E.get("pa_bufs", 1)), space="PSUM")
        )
        pbp = ctx.enter_context(
            tc.tile_pool(name="pbp", bufs=int(E.get("pb_bufs", 2)), space="PSUM")
        )
        p4p = ctx.enter_context(tc.tile_pool(name="p4p", bufs=2, space="PSUM"))

        # ---- load constants into SBUF
        x_sb = const.tile([OUT, BC], f32r, name="x_sb")
        win = const.tile([OUT, HIDDEN], f32r, name="win")
        w1 = const.tile([P, 512], f32r, name="w1")
        w2 = const.tile([P, 512], f32r, name="w2")
        wout = const.tile([P, 128], f32r, name="wout")
        ident = const.tile([P, 384], f32r, name="ident")
        bia = const.tile([P, 10], f32, name="bia")
        b2row = const.tile([1, 256], f32r, name="b2row")
        onerow = const.tile([1, 256], f32r, name="onerow")
        nc.sync.dma_start(x_sb[:], xT[:, :])
        nc.sync.dma_start(win[:], winT_d[:, :])
        nc.sync.dma_start(w1[:], w1T_d[:, :])
        nc.sync.dma_start(w2[:], w2T_d[:, :])
        nc.sync.dma_start(wout[:], woutT_d[:, :])
        nc.sync.dma_start(ident[:], ident_d[:, :])
        nc.sync.dma_start(bia[:], bias_d[:, :])
        nc.sync.dma_start(b2row[:], b2r_d[:, :])
        nc.sync.dma_start(onerow[:], one_d[:, :])

        # PE matmuls may carry at most ONE sync wait; absorb every const-DMA
        # queue tick into the PE vector clock up front via dummy 1x1 matmuls.
        dmy = pa.tile([1, 1], f32, tag="pA0", name="dmy")
        for cst in (x_sb, win, w1, w2, wout, ident, bia, b2row, onerow):
            c1 = cst[:, 0:1].bitcast(f32)  # f32r 1x1 matmul is invalid ISA
            nc.tensor.matmul(
                dmy[:], c1, c1, start=True, stop=True, skip_group_check=True
            )

        I2 = ident[:, 0:128]  # (2/dtm) I
        I4 = ident[:, 128:256]  # (4/dtm) I

        def bcol(j):  # [128,1] bias column
            return bia[:, j : j + 1]

        # bias cols (q = W1@b2): 0,1 b_in; 2,3 b1 (z@j0);
        # 4,5 b1+(dtm/2)q (z@j1,j2); 6,7 dtm*b2 (s evict); 8,9 b1+dtm*q (z@j3)
        ZB = (8, 4, 4, 8)  # z bias col base per stage (j0 reads hr' = h - dt*b2 comp)

        def wblk(w, k, m):  # W1T/W2T block (k, m)
            j = (k * 2 + m) * 128
            return w[:, j : j + 128]

        def new_h(b):
            return hpool.tile([P, 2 * HB], f32, tag=f"hb{b}", name="h")

        def new_hr(b):
            return hrpool.tile([P, 2 * HB], f32r, tag=f"hrb{b}", name="hr")

        def kv(hh_b, k):  # k-chunk view of a per-half tile
            return hh_b[:, k * HB : (k + 1) * HB]

        tte = {"dve": nc.vector, "gps": nc.gpsimd}

        # ---- h0 = W_in @ xT + b_in   (full batch, N=512)
        h = [new_h(0), new_h(1)]
        hr = [new_hr(0), new_hr(1)]
        for m in range(2):
            ps = pa.tile([P, BC], f32, tag=f"pA{m}", name="ps_init")
            nc.tensor.matmul(
                ps[:], win[:, m * 128 : (m + 1) * 128], x_sb[:], start=True, stop=True
            )
            for b in range(2):
                src = ps[:, b * HB : (b + 1) * HB]
                if b == 0:
                    nc.scalar.activation(
                        kv(h[b], m), src, AF.Identity, bias=bcol(m), scale=1.0
                    )
                else:
                    nc.vector.tensor_scalar(
                        kv(h[b], m), src, bcol(m), None, op0=ALU.add
                    )
            for b in range(2):
                src2 = ps[:, b * HB : (b + 1) * HB]
                nc.vector.tensor_scalar(
                    kv(hr[b], m), src2, bcol(2 + m), None, op0=ALU.add
                )

        def emit_op_mm(h_b, b):
            pso = p4p.tile([OUT, HB], f32, tag="p4", name="pso")
            for k in range(2):
                nc.tensor.matmul(
                    pso[:], wout[:, k * 64 : (k + 1) * 64], kv(h_b, k),
                    start=(k == 0), stop=(k == 1),
                )
            return pso

        def emit_osb(t, pso, b):
            # pso eviction + DMA, deferred so it never sits ahead of
            # latency-critical z/tmp ops in the ACT/DVE queues
            osb = opool.tile([OUT, HB], f32, tag=f"osb{b}", name="osb")
            if E[f"osb_b{b}"] == "act":
                nc.scalar.copy(osb[:], pso[:])
            else:
                nc.vector.tensor_copy(osb[:], pso[:])
            nc.sync.dma_start(out_d[t, :, b * HB : (b + 1) * HB], osb[:])

        def emit_outproj(t, h_b, b):
            emit_osb(t, emit_op_mm(h_b, b), b)

        def emit_L1(b, iv):
            pA = [
                pa.tile([P, HB], f32, tag="pA0", name="pAt"),
                pa.tile([P, HB], f32, tag="pA1", name="pAt"),
            ]
            for k, m in ((0, 0), (0, 1), (1, 0), (1, 1)):
                nc.tensor.matmul(
                    pA[m][:], wblk(w1, k, m), iv[k],
                    start=(k == 0), stop=(k == 1),
                    skip_group_check=True,
                )
            return pA

        def emit_z(b, j, pA):
            z = [
                zpool.tile([P, HB], f32r, tag=f"z{j}0", name="z0"),
                zpool.tile([P, HB], f32r, tag=f"z{j}1", name="z1"),
            ]
            for m in range(2):
                engn = E.get(f"z_j{j}_b{b}m{m}", E[f"z_b{b}m{m}"])
                bc = bcol(ZB[j] + m)
                if engn == "act":
                    nc.scalar.activation(
                        z[m][:], pA[m][:], AF.Relu, bias=bc, scale=1.0
                    )
                else:
                    nc.vector.tensor_scalar(
                        z[m][:], pA[m][:], bc, 0.0, op0=ALU.add, op1=ALU.max
                    )
            return z

        def emit_L2(b, z):
            pB = [
                pbp.tile([P, HB], f32, tag="pB0", name="pBt"),
                pbp.tile([P, HB], f32, tag="pB1", name="pBt"),
            ]
            for k, m in ((0, 0), (0, 1), (1, 0), (1, 1)):
                nc.tensor.matmul(
                    pB[m][:], wblk(w2, k, m), z[k][:],
                    start=(k == 0), stop=(k == 1),
                    skip_group_check=True,
                )
            return pB

        def emit_tmp(b, j, pB, dt):
            # tmp = c_j*(W2@z_j) + h in ONE DVE op from PSUM (critical path)
            c = (dt / 2.0, dt / 2.0, dt)[j]
            tmp = [
                tpool.tile([P, HB], f32r, tag="tmp0", name="t0"),
                tpool.tile([P, HB], f32r, tag="tmp1", name="t1"),
            ]
            for m in range(2):
                nc.vector.scalar_tensor_tensor(
                    tmp[m][:], pB[m][:], c, kv(h[b], m).bitcast(f32r),
                    op0=ALU.mult, op1=ALU.add,
                )
            return tmp

        def emit_acc1(b, zs):
            # zsum123 = z1+2z2+2z3 = (z1+z2) + (2z3+z2), spread over stages:
            # acc1 = z1+z2 at j1 (plain add, idle GPSIMD), acc2 = 2z3+z2 at
            # j2 (one DVE stt), final add on GPSIMD. Exact RK4 weights.
            acc = []
            for m in range(2):
                a = zspool.tile([P, HB], f32r, tag=f"a1{m}", name="a1")
                tte[E["acc1"]].tensor_tensor(
                    a[:], zs[0][b][m][:], zs[1][b][m][:], op=ALU.add
                )
                acc.append(a)
            return acc

        def emit_acc2(b, zs):
            acc = []
            for m in range(2):
                a = zspool.tile([P, HB], f32r, tag=f"a2{m}", name="a2")
                nc.vector.scalar_tensor_tensor(
                    a[:], zs[2][b][m][:], 2.0, zs[1][b][m][:],
                    op0=ALU.mult, op1=ALU.add,
                )
                acc.append(a)
            return acc

        def emit_zsum(b, acc1, acc2):
            zsum = []
            for m in range(2):
                zsm = zspool.tile([P, HB], f32r, tag=f"zs{m}", name="zsm")
                tte[E["zsum"]].tensor_tensor(
                    zsm[:], acc1[b][m][:], acc2[b][m][:], op=ALU.add
                )
                zsum.append(zsm)
            return zsum

        def emit_w2grp(b, zin, pb4, first):
            # 4-matmul W2 group accumulating into pb4[b]
            for k, m in ((0, 0), (0, 1), (1, 0), (1, 1)):
                nc.tensor.matmul(
                    pb4[b][m][:], wblk(w2, k, m), zin[k][:],
                    start=(first and k == 0 and m == 0),
                    stop=(not first and k == 1 and m == 1),
                    skip_group_check=True,
                )

        def emit_s_hn(b, hn, hrn, h):
            sc = dtm / 6.0
            # hr' = h + sc*pb4 (one stt straight from PSUM, first in queue:
            # shortest path to the next step's L1/outproj). The missing
            # dt*b2 term is compensated exactly in hr's consumers: the j0
            # L1 bias column (b1 + dtm*W1@b2) and a host-side constant on
            # b_out. The strict-f32 state hn keeps the exact s path.
            for kq in range(2):
                nc.vector.scalar_tensor_tensor(
                    kv(hrn[b], kq), pb4_cur[b][kq][:], sc,
                    kv(h[b], kq).bitcast(f32r), op0=ALU.mult, op1=ALU.add,
                )
            sl = [
                spool.tile([P, HB], f32, tag="s0", name="s0"),
                spool.tile([P, HB], f32, tag="s1", name="s1"),
            ]
            for m in range(2):
                if E.get(f"s_b{b}m{m}", E[f"s_b{b}"]) == "act":
                    nc.scalar.activation(
                        sl[m][:], pb4_cur[b][m][:], AF.Identity,
                        bias=bcol(6 + m), scale=sc,
                    )
                else:
                    nc.vector.tensor_scalar(
                        sl[m][:], pb4_cur[b][m][:], sc, bcol(6 + m),
                        op0=ALU.mult, op1=ALU.add,
                    )
            for kq in range(2):
                tte[E[f"hn_k{kq}"]].tensor_tensor(
                    kv(hn[b], kq), kv(h[b], kq), sl[kq][:], op=ALU.add
                )

        # ---- time stepping (repeat>1 is a timing-only mode)
        for t in range(nsteps * repeat):
            dt = dts[t % nsteps]
            hn = [new_h(0), new_h(1)]
            hrn = [new_hr(0), new_hr(1)]
            inp = [
                [kv(hr[0], 0), kv(hr[0], 1)],
                [kv(hr[1], 0), kv(hr[1], 1)],
            ]
            pb4 = [None, None]
            pb4_cur = pb4

            pso = [None, None]
            zs = [[None, None] for _ in range(4)]  # zs[j][b]
            acc1 = [None, None]
            acc2 = [None, None]
            zsum = [None, None]
            for j in range(3):  # RK4 stages 0-2
                # alt_halves: swap which half leads on odd stages so the
                # trailing-half latency penalty alternates instead of
                # always hitting b1
                ba, bb = (1, 0) if (E.get("alt_halves", 0) and j % 2) else (0, 1)
                pAa = emit_L1(ba, inp[ba])
                zs[j][ba] = emit_z(ba, j, pAa)
                # -- slot after L1(ba)+z: guaranteed-ready PE work
                if j == 0 and not E.get("op_at_j1", 0):
                    pso[ba] = emit_op_mm(hr[ba], ba)
                    if E["opb1_pos"] == "early":
                        pso[bb] = emit_op_mm(hr[bb], bb)

                pAb = emit_L1(bb, inp[bb])
                zs[j][bb] = emit_z(bb, j, pAb)
                if j == 1 and E.get("op_at_j1", 0):
                    pso[0] = emit_op_mm(hr[0], 0)
                    pso[1] = emit_op_mm(hr[1], 1)

                pBa = emit_L2(ba, zs[j][ba])
                tmpa = emit_tmp(ba, j, pBa, dt)
                inp[ba] = [tmpa[0][:], tmpa[1][:]]
                if j == 2:
                    # acc2 behind tmp in the DVE queue: tmp is critical
                    acc2[ba] = emit_acc2(ba, zs)
                # -- slot after L2(ba)
                if j == 0 and not E.get("op_at_j1", 0) and E["opb1_pos"] == "mid":
                    pso[bb] = emit_op_mm(hr[bb], bb)
                if j == 2:
                    zsum[ba] = emit_zsum(ba, acc1, acc2)
                pBb = emit_L2(bb, zs[j][bb])
                tmpb = emit_tmp(bb, j, pBb, dt)
                inp[bb] = [tmpb[0][:], tmpb[1][:]]
                if j == 0 and not E.get("op_at_j1", 0) and E["opb1_pos"] == "end":
                    pso[bb] = emit_op_mm(hr[bb], bb)
                if j == 1:
                    # deferred pso evictions: quiet queue point
                    emit_osb(t % nsteps, pso[0], 0)
                    emit_osb(t % nsteps, pso[1], 1)
                    acc1[0] = emit_acc1(0, zs)
                    acc1[1] = emit_acc1(1, zs)
                if j == 2:
                    acc2[bb] = emit_acc2(bb, zs)
                    zsum[bb] = emit_zsum(bb, acc1, acc2)

            # ---- stage 3: W2 groups accumulate pb4, then s/hr/hn
            pA0 = emit_L1(0, inp[0])
            zs[3][0] = emit_z(0, 3, pA0)
            for b in range(2):
                pt = p4p.tile([P, 2 * HB], f32, tag="p4", name="pb4")
                pb4[b] = [pt[:, 0:HB], pt[:, HB : 2 * HB]]
            pb4_cur = pb4
            emit_w2grp(0, zsum[0], pb4, first=True)
            if E.get("j3_order", "abab") == "aabb":
                # b0's full boundary chain first: its s->hr starts earliest
                emit_w2grp(0, zs[3][0], pb4, first=False)
                emit_s_hn(0, hn, hrn, h)
                pA1 = emit_L1(1, inp[1])
                zs[3][1] = emit_z(1, 3, pA1)
                emit_w2grp(1, zsum[1], pb4, first=True)
                emit_w2grp(1, zs[3][1], pb4, first=False)
                emit_s_hn(1, hn, hrn, h)
            else:
                pA1 = emit_L1(1, inp[1])
                zs[3][1] = emit_z(1, 3, pA1)
                emit_w2grp(0, zs[3][0], pb4, first=False)
                emit_s_hn(0, hn, hrn, h)
                emit_w2grp(1, zsum[1], pb4, first=True)
                emit_w2grp(1, zs[3][1], pb4, first=False)
                emit_s_hn(1, hn, hrn, h)
            h = hn
            hr = hrn

        # final output projection (t = nsteps)
        for b in range(2):
            emit_outproj(nsteps, hr[b], b)

    nc.compile()  # bacc passes: event-sem legalization, reg alloc, DCE
    return nc


def _prep_shared(W_in, b_in, W1, b1, W2, b2, W_out, dtm):
    f = np.float32

    def pack_blocks(WT):  # [256,256] -> [128, 512] blocks (k*2+m)
        blks = [
            WT[k * 128 : (k + 1) * 128, m * 128 : (m + 1) * 128]
            for k in range(2)
            for m in range(2)
        ]
        return np.ascontiguousarray(np.concatenate(blks, axis=1), dtype=f)

    winT = np.ascontiguousarray(W_in.T, dtype=f)  # [64, 256]
    w1T = pack_blocks(W1.T.astype(f))
    w2T = pack_blocks(W2.T.astype(f))
    wt = W_out.T.astype(f)  # [256, 64]
    woutT = np.ascontiguousarray(
        np.concatenate([wt[0:128, :], wt[128:256, :]], axis=1), dtype=f
    )  # [128, 128]

    I = np.eye(128, dtype=f)
    ident = np.ascontiguousarray(
        np.concatenate(
            [(f(2.0) / dtm) * I, (f(4.0) / dtm) * I, (f(6.0) / dtm) * I], axis=1
        ),
        dtype=f,
    )

    def cols2(v):  # [256] -> two [128] cols
        return [v[0:128], v[128:256]]

    b1 = b1.astype(f)
    b2 = b2.astype(f)
    q = (W1.astype(f) @ b2).astype(f)  # W1 @ b2, [256]
    cols = (
        cols2(b_in.astype(f))
        + cols2(b_in.astype(f) - dtm * b2)  # init hr' = h0 - dtm*b2
        + cols2(b1 + (dtm / f(2.0)) * q)  # z bias @ j1, j2
        + cols2(dtm * b2)                 # s-evict bias
        + cols2(b1 + dtm * q)             # z bias @ j3
    )
    biases = np.ascontiguousarray(np.stack(cols, axis=1), dtype=f)  # [128, 10]
    b2row = np.ascontiguousarray((f(6.0) * b2).reshape(1, 256), dtype=f)
    onerow = np.ones((1, 256), dtype=f)
    return dict(winT=winT, w1T=w1T, w2T=w2T, woutT=woutT, ident=ident,
                biases=biases, b2row=b2row, onerow=onerow)


_last_results = None


def kernel(x, t_span, W_in, b_in, W1, b1, W2, b2, W_out, b_out):
    global _last_results
    from concourse.bass_utils import run_bass_kernel_spmd

    f = np.float32
    x = np.asarray(x, f)
    t_span = np.asarray(t_span, f)
    dts = np.diff(t_span).astype(f)
    dtm = f(dts.mean())

    key = dts.tobytes()
    if key not in _cache:
        _cache[key] = _build([float(d) for d in dts], float(dtm))
    nc = _cache[key]

    shared = _prep_shared(
        np.asarray(W_in), np.asarray(b_in), np.asarray(W1), np.asarray(b1),
        np.asarray(W2), np.asarray(b2), np.asarray(W_out), dtm,
    )
    in_maps = []
    for c in range(NCORES):
        xc = np.ascontiguousarray(x[c * BC : (c + 1) * BC].T, dtype=f)  # [64, 512]
        m = dict(shared)
        m["xT"] = xc
        in_maps.append(m)

    res = run_bass_kernel_spmd(nc, in_maps, core_ids=list(range(NCORES)))
    _last_results = res
    outs = [np.asarray(r["out"]) for r in res.results]  # each [100, 64, 512]
    full = np.concatenate([o.transpose(0, 2, 1) for o in outs], axis=1)
    # hr' carries h - dt*b2; compensate the outproj exactly
    bo = np.asarray(b_out, f) + dtm * (np.asarray(W_out, f) @ np.asarray(b2, f))
    full = full + bo[None, None, :]
    return np.ascontiguousarray(full, dtype=f)
